# revision 13
# baseline (speedup 1.0000x reference)
"""Trainium2 Bass kernel for Bahdanau additive attention (nn_AttentionLayer).

Reference math (per batch b; t_q=128, t_k=512, n=512, h=128):
    q_proj = query @ Wq.T + bq                    # [t_q, h]
    k_proj = keys  @ Wk.T + bk                    # [t_k, h]
    scores[i,j] = Wo[0] . tanh(q_proj[i] + k_proj[j]) (+ bo, softmax-invariant)
    attn = softmax(scores, axis=-1)
    context = attn @ values
    returns (context, attn)

Sharding: data-parallel over batch b — one batch element per NeuronCore (8 cores).

Device layout strategy (per core):
  * kpT[h=128, j=512] = Wk @ keys.T held with hidden dim on partitions.
  * qpb[h=128, i=128] = Wq @ query.T + (bq+bk) — per-query bias columns.
  * Main loop over 128 queries: one ScalarE activation
        hid_i[h, j] = tanh(kpT + qpb[:, i])   (add fused into the ACT bias)
    then one TensorE matmul with a zero-padded stationary weight
        lhsT = wo_shift[:, i%32, :] ([h=128, 32]; Wo in column i%32, zeros else)
    accumulating scores directly into PSUM rows of a single [128, 512] scores
    tile — scores come out in the natural [i, j] layout.
  * Softmax: Exp with accum_out (free-dim row-sum) -> reciprocal -> scale.
  * context = attn @ values via 4 PE transposes of attn + 4 accumulated matmuls.
"""

from contextlib import ExitStack

import numpy as np

import concourse.bass as bass
import concourse.tile as tile
from concourse import bacc, masks, mybir
from concourse.bass_utils import run_bass_kernel_spmd

F32 = mybir.dt.float32
AF = mybir.ActivationFunctionType

B = 8          # batch (== number of cores)
TQ = 128       # query positions
TK = 512       # key positions
NQ = 512       # query feature dim
NK = 512       # key feature dim
NV = 512       # value feature dim
H = 128        # hidden dim
STRIP = 32     # query strip width (PE column-group granularity)

_CACHE: dict = {}


def _build_nc() -> bass.Bass:
    nc = bacc.Bacc("TRN2", target_bir_lowering=False, debug=False)

    q_d = nc.dram_tensor("query", [TQ, NQ], F32, kind="ExternalInput")
    k_d = nc.dram_tensor("keys", [TK, NK], F32, kind="ExternalInput")
    v_d = nc.dram_tensor("values", [TK, NV], F32, kind="ExternalInput")
    wqt_d = nc.dram_tensor("WqT", [NQ, H], F32, kind="ExternalInput")
    wkt_d = nc.dram_tensor("WkT", [NK, H], F32, kind="ExternalInput")
    bqk_d = nc.dram_tensor("bqk", [H, 1], F32, kind="ExternalInput")
    wosh_d = nc.dram_tensor("wo_shift", [H, STRIP, STRIP], F32, kind="ExternalInput")
    ctx_d = nc.dram_tensor("context", [TQ, NV], F32, kind="ExternalOutput")
    attn_d = nc.dram_tensor("attn", [TQ, TK], F32, kind="ExternalOutput")

    KC = NK // 128  # 4 contraction chunks over the feature dim
    JC = TK // 128  # 4 chunks over key positions

    with tile.TileContext(nc) as tc:
        with ExitStack() as ctx:
            consts = ctx.enter_context(tc.tile_pool(name="consts", bufs=1))
            ins = ctx.enter_context(tc.tile_pool(name="ins", bufs=1))
            tp_ps = ctx.enter_context(
                tc.tile_pool(name="tp_ps", bufs=2, space=bass.MemorySpace.PSUM)
            )
            proj_ps = ctx.enter_context(
                tc.tile_pool(name="proj_ps", bufs=1, space=bass.MemorySpace.PSUM)
            )
            score_ps = ctx.enter_context(
                tc.tile_pool(name="score_ps", bufs=1, space=bass.MemorySpace.PSUM)
            )
            ctx_ps = ctx.enter_context(
                tc.tile_pool(name="ctx_ps", bufs=1, space=bass.MemorySpace.PSUM)
            )
            hid_pool = ctx.enter_context(tc.tile_pool(name="hid", bufs=3))
            sm_pool = ctx.enter_context(tc.tile_pool(name="sm", bufs=1))
            att_pool = ctx.enter_context(tc.tile_pool(name="attT", bufs=2))

            # ---- constants / inputs ----
            with nc.named_scope("load"):
                ident = consts.tile([128, 128], F32, tag="ident")
                masks.make_identity(nc, ident[:])

                wosh = consts.tile([H, STRIP, STRIP], F32, tag="wosh")
                nc.sync.dma_start(wosh[:], wosh_d.ap())
                bqk = consts.tile([H, 1], F32, tag="bqk")
                nc.sync.dma_start(bqk[:], bqk_d.ap())
                wqt = consts.tile([128, KC, H], F32, tag="wqt")
                nc.sync.dma_start(
                    wqt[:], wqt_d.ap().rearrange("(c p) h -> p c h", p=128)
                )
                wkt = consts.tile([128, KC, H], F32, tag="wkt")
                nc.sync.dma_start(
                    wkt[:], wkt_d.ap().rearrange("(c p) h -> p c h", p=128)
                )

                k_nat = []
                for r in range(JC):
                    t = ins.tile([128, NK], F32, tag=f"k_nat{r}")
                    nc.sync.dma_start(t[:], k_d.ap()[r * 128 : (r + 1) * 128, :])
                    k_nat.append(t)
                q_nat = ins.tile([TQ, NQ], F32, tag="q_nat")
                nc.sync.dma_start(q_nat[:], q_d.ap())
                v_sb = []
                for c in range(JC):
                    t = ins.tile([128, NV], F32, tag=f"v{c}")
                    nc.sync.dma_start(t[:], v_d.ap()[c * 128 : (c + 1) * 128, :])
                    v_sb.append(t)

            # ---- transpose keys and query (feature dim -> partitions) ----
            with nc.named_scope("transpose"):
                kT = []  # kT[c][p, j] = keys[j, c*128+p]
                for c in range(KC):
                    t = ins.tile([128, TK], F32, tag=f"kT{c}")
                    kT.append(t)
                    for r in range(JC):
                        pst = tp_ps.tile([128, 128], F32, tag="tpp")
                        nc.tensor.transpose(
                            pst[:], k_nat[r][:, c * 128 : (c + 1) * 128], ident[:]
                        )
                        nc.vector.tensor_copy(t[:, r * 128 : (r + 1) * 128], pst[:])
                qT = []
                for c in range(KC):
                    t = ins.tile([128, TQ], F32, tag=f"qT{c}")
                    pst = tp_ps.tile([128, 128], F32, tag="tpp")
                    nc.tensor.transpose(
                        pst[:], q_nat[:, c * 128 : (c + 1) * 128], ident[:]
                    )
                    nc.vector.tensor_copy(t[:], pst[:])
                    qT.append(t)

            # ---- projections ----
            with nc.named_scope("proj"):
                kpT_ps = proj_ps.tile([H, TK], F32, tag="kpT")  # persistent PSUM bank
                for c in range(KC):
                    nc.tensor.matmul(
                        kpT_ps[:],
                        wkt[:, c, :],
                        kT[c][:],
                        start=(c == 0),
                        stop=(c == KC - 1),
                    )
                qp_ps = proj_ps.tile([H, TQ], F32, tag="qp")
                for c in range(KC):
                    nc.tensor.matmul(
                        qp_ps[:],
                        wqt[:, c, :],
                        qT[c][:],
                        start=(c == 0),
                        stop=(c == KC - 1),
                    )
                qpb = consts.tile([H, TQ], F32, tag="qpb")
                nc.scalar.activation(qpb[:], qp_ps[:], AF.Identity, bias=bqk[:, 0:1])

            # ---- scores: loop over queries ----
            # ST[i, j] accumulates scores in natural layout via zero-padded weights.
            with nc.named_scope("scores"):
                st = score_ps.tile([TQ, TK], F32, tag="st")
                nstrips = TQ // STRIP
                for s in range(nstrips):
                    for q in range(STRIP):
                        i = s * STRIP + q
                        hid = hid_pool.tile([H, TK], F32, tag="hid")
                        nc.scalar.activation(
                            hid[:], kpT_ps[:], AF.Tanh, bias=qpb[:, i : i + 1]
                        )
                        nc.tensor.matmul(
                            st[s * STRIP : (s + 1) * STRIP, :],
                            wosh[:, q, :],
                            hid[:],
                            start=(q == 0),
                            stop=(q == STRIP - 1),
                            tile_position=(0, s * STRIP),
                        )

            # ---- softmax (no max-subtraction needed: |scores| <= ~12) ----
            with nc.named_scope("softmax"):
                exp_sb = sm_pool.tile([TQ, TK], F32, tag="exp")
                denom = sm_pool.tile([TQ, 1], F32, tag="denom")
                nc.scalar.activation(exp_sb[:], st[:], AF.Exp, accum_out=denom[:])
                recip = sm_pool.tile([TQ, 1], F32, tag="recip")
                nc.vector.reciprocal(recip[:], denom[:])
                attn_sb = sm_pool.tile([TQ, TK], F32, tag="attn")
                nc.vector.tensor_scalar_mul(attn_sb[:], exp_sb[:], recip[:, 0:1])
                nc.sync.dma_start(attn_d.ap(), attn_sb[:])

            # ---- context = attn @ values ----
            with nc.named_scope("context"):
                attT = []
                for c in range(JC):
                    pst = tp_ps.tile([128, 128], F32, tag="tpp")
                    nc.tensor.transpose(
                        pst[:], attn_sb[:, c * 128 : (c + 1) * 128], ident[:]
                    )
                    t = att_pool.tile([128, TQ], F32, tag="attT")
                    nc.vector.tensor_copy(t[:], pst[:])
                    attT.append(t)
                cps = ctx_ps.tile([TQ, NV], F32, tag="ctx")
                for c in range(JC):
                    nc.tensor.matmul(
                        cps[:],
                        attT[c][:],
                        v_sb[c][:],
                        start=(c == 0),
                        stop=(c == JC - 1),
                    )
                ctx_sb = sm_pool.tile([TQ, NV], F32, tag="ctx_sb")
                nc.vector.tensor_copy(ctx_sb[:], cps[:])
                nc.sync.dma_start(ctx_d.ap(), ctx_sb[:])

    nc.finalize()
    return nc


def _get_nc() -> bass.Bass:
    if "nc" not in _CACHE:
        _CACHE["nc"] = _build_nc()
    return _CACHE["nc"]


def _prep_in_maps(query, keys, values, Wq, bq, Wk, bk, Wo, bo):
    WqT = np.ascontiguousarray(np.asarray(Wq, np.float32).T)
    WkT = np.ascontiguousarray(np.asarray(Wk, np.float32).T)
    bqk = (np.asarray(bq, np.float32) + np.asarray(bk, np.float32)).reshape(H, 1)
    wo_shift = np.zeros((H, STRIP, STRIP), np.float32)
    idx = np.arange(STRIP)
    wo_shift[:, idx, idx] = np.asarray(Wo, np.float32)[0][:, None]
    wo_shift = np.ascontiguousarray(wo_shift)
    in_maps = []
    for b in range(B):
        in_maps.append(
            {
                "query": np.ascontiguousarray(query[b], dtype=np.float32),
                "keys": np.ascontiguousarray(keys[b], dtype=np.float32),
                "values": np.ascontiguousarray(values[b], dtype=np.float32),
                "WqT": WqT,
                "WkT": WkT,
                "bqk": bqk,
                "wo_shift": wo_shift,
            }
        )
    return in_maps


def _run(inputs: dict, trace: bool = False):
    nc = _get_nc()
    in_maps = _prep_in_maps(**inputs)
    try:
        res = run_bass_kernel_spmd(nc, in_maps, core_ids=list(range(B)), trace=trace)
    except Exception:
        if not trace:
            raise
        import traceback

        traceback.print_exc()
        print("trace run failed; falling back to untraced run")
        res = run_bass_kernel_spmd(nc, in_maps, core_ids=list(range(B)), trace=False)
    context = np.stack([res.results[b]["context"] for b in range(B)])
    attn = np.stack([res.results[b]["attn"] for b in range(B)])
    return (context, attn), res


def kernel(**inputs):
    (context, attn), _ = _run(inputs, trace=False)
    return context, attn
